# revision 19
# baseline (speedup 1.0000x reference)
"""Trainium2 Bass kernel for the brute-force antisymmetrized ResNet.

Math (per walker b):
    feats[i,j] = concat(x1[P1[i]], x2[P2[j]]).reshape(24)    (576 = 24*24 perm pairs)
    y0 = tanh(feats @ W0 + b0)
    y1 = tanh(y0 @ W1 + b1) + y0
    y2 = tanh(y1 @ W2 + b2) + y1
    out[b] = log| sum_{i,j} s1[i] s2[j] (y2 @ Wf + bf) |

Strategy:
  - Data-parallel over the 512 walkers: 64 walkers per NeuronCore x 8 cores.
  - First layer is factored: y0pre[b,i,j] = u1[b,i] + u2[b,j] where
    u1[b,i] = x1feats(perm i) @ W0[:12] + b0 and u2[b,j] = x2feats(perm j) @
    W0[12:24].  The device computes u1/u2 with tiny matmuls and
    broadcast-adds them into the 576 rows per walker on VectorE.
  - Permutations are sign-sorted into quadrants so each walker's 576 rows are
    ordered [(+,+), (-,-), (+,-), (-,+)] x 12 x 12: the first 288 rows have
    pair-sign +1, the last 288 have -1.
  - Plain fp32 matmuls (fp32r/tf32 rounding is amplified catastrophically by
    the antisymmetrization: |anti| reaches 2.6e-4 against O(1) terms).
  - Instruction-count-minimized dataflow (~1065 instructions vs ~1980 for
    the per-piece predecessor): fixed 4-bank PSUM regions (ps1 for layer 1,
    ps2 for layer 2) let tanh1/tanh2 each cover TWO 512-row tiles in a
    single ScalarE instruction; tanh0 is 3 instructions per 4608-row group
    (in the h0 ring); inputs arrive in 3 packed DMAs.
  - The sign-segment reduction is done as per-32-row block sums
    (32 = gcd(512, 288), so blocks never straddle walker/sign boundaries)
    with ONE 4D-AP VectorE reduce per 2048-row window per tensor; the host
    combines block sums in fp64 and applies Wf and log|.|.  The short fp32
    accumulation chains cut max rel err ~6x vs 288-row device sums.
  - sum(y2) = sum(t1) + sum(tanh2) per block (y2 = tanh2 + t1), so only
    those two tensors are reduced on device.
  - Engine split: PE matmuls; ScalarE tanh; VectorE broadcast-adds +
    block reduces; GpSimd the t1 residual adds, so the latency-critical
    tanh1 -> resid -> layer-2 edge never queues behind VectorE work.
    The next group's h0 bcasts/tanh0 are spread over tiles sg 0..4 so the
    chain completes well before the group boundary.
"""

import itertools

import numpy as np

N1 = 4
N2 = 4
D = 3
BATCH = 512
NDENSE = 256
NCORES = 8
NPERM = 24              # 4!
NPAIR = NPERM * NPERM   # 576
HALFPAIR = NPAIR // 2   # 288 rows of each sign per walker
QUAD = NPERM // 2 * (NPERM // 2)            # 144 rows per quadrant
WALKERS_PER_CORE = BATCH // NCORES          # 64
ROWS_PER_CORE = WALKERS_PER_CORE * NPAIR    # 36864
TILE = 512                                  # matmul moving-dim tile
GROUP_WALKERS = 8                           # walkers per h0-ring group
GROUP_ROWS = GROUP_WALKERS * NPAIR          # 4608 = 9 * TILE
TILES_PER_GROUP = GROUP_ROWS // TILE        # 9
NGROUPS = ROWS_PER_CORE // GROUP_ROWS       # 8
NTILES = NGROUPS * TILES_PER_GROUP          # 72
NSUPER = NTILES // 2                        # 36 psum supertiles (1024 rows)
NWIN = NTILES // 4                          # 18 reduce windows (2048 rows)
WINROWS = 4 * TILE                          # 2048
BLK = 32                                    # gcd(TILE, HALFPAIR)
BPW = WINROWS // BLK                        # 64 blocks per window
WCOLS = 2 * 2 * BPW                         # vout cols per window: l, h, b
VCOLS = NWIN * WCOLS                        # 4608
UCOLS = WALKERS_PER_CORE * NPERM            # 1536 u-columns per core
K1 = N1 * D + 1                             # 13: x1 features + ones row (b0)
K2 = N2 * D                                 # 12


def _perms_and_signs(n):
    P = np.array(list(itertools.permutations(range(n))), dtype=np.int32)
    triu = np.triu(np.ones((n, n), dtype=np.int64), 1)
    inv = np.sum((P[:, :, None] > P[:, None, :]) * triu, axis=(1, 2))
    signs = np.where(inv % 2 == 0, 1.0, -1.0).astype(np.float32)
    return P, signs


_P1, _S1 = _perms_and_signs(N1)
_P2, _S2 = _perms_and_signs(N2)

# sign-sorted perm orders: 12 even perms then 12 odd perms
_ISORT = np.concatenate([np.where(_S1 > 0)[0], np.where(_S1 < 0)[0]])
_JSORT = np.concatenate([np.where(_S2 > 0)[0], np.where(_S2 < 0)[0]])
# quadrants (a, b): pair-sign = +1 for the first two, -1 for the last two
_QUADS = [(0, 0), (1, 1), (0, 1), (1, 0)]

_cached = {}
_last_results = None  # BassKernelResults of the most recent run (for profiling)


def _build_nc(with_bias: bool):
    """Build + compile the 8-core SPMD Tile kernel (cached)."""
    key = bool(with_bias)
    if key in _cached:
        return _cached[key]

    import concourse.bacc as bacc
    import concourse.tile as tile
    from concourse import mybir

    FP = mybir.dt.float32
    TANH = mybir.ActivationFunctionType.Tanh
    AXX = mybir.AxisListType.X

    nc = bacc.Bacc(
        "TRN2",
        target_bir_lowering=False,
        debug=False,
        num_devices=NCORES,
    )

    # packed inputs: one DMA each for perm-features, first-layer weights,
    # and the four 128-row halves of W1/W2
    # x2 rows sit at partition 32 (matmul base partition must be 0/32/64)
    xf_d = nc.dram_tensor("xf", [32 + K2, UCOLS], FP, kind="ExternalInput").ap()
    xw_d = nc.dram_tensor("xw", [32 + K2, NDENSE], FP, kind="ExternalInput").ap()
    w_d = nc.dram_tensor("w", [128, 4 * NDENSE], FP, kind="ExternalInput").ap()
    if with_bias:
        b_d = nc.dram_tensor("b12", [128, 4], FP, kind="ExternalInput").ap()
    v_d = nc.dram_tensor("v", [128, VCOLS], FP, kind="ExternalOutput").ap()

    with tile.TileContext(nc) as tc:
        with (
            tc.tile_pool(name="consts", bufs=1) as cpool,
            tc.tile_pool(name="t1p", bufs=2) as t1pool,
            tc.tile_pool(name="t2p", bufs=2) as t2pool,
            tc.tile_pool(name="h0ring", bufs=2) as hpool,
            tc.tile_pool(name="vout", bufs=1) as vpool,
            tc.tile_pool(name="ps", bufs=1, space="PSUM") as pspool,
        ):
            xf = cpool.tile([32 + K2, UCOLS], FP, tag="xf")
            nc.sync.dma_start(xf[:], xf_d[:])
            xw = cpool.tile([32 + K2, NDENSE], FP, tag="xw")
            nc.sync.dma_start(xw[:], xw_d[:])
            wsb = cpool.tile([128, 4 * NDENSE], FP, tag="wsb")
            nc.sync.dma_start(wsb[:], w_d[:])
            x1f, x2f = xf[0:K1, :], xf[32:32 + K2, :]
            x1w, x2w = xw[0:K1, :], xw[32:32 + K2, :]
            w1a = wsb[:, 0 * NDENSE:1 * NDENSE]
            w1b = wsb[:, 1 * NDENSE:2 * NDENSE]
            w2a = wsb[:, 2 * NDENSE:3 * NDENSE]
            w2b = wsb[:, 3 * NDENSE:4 * NDENSE]
            if with_bias:
                bsb = cpool.tile([128, 4], FP, tag="b12")  # b1h0 b1h1 b2h0 b2h1
                nc.sync.dma_start(bsb[:], b_d[:])

            u1s = cpool.tile([128, 2, UCOLS], FP, tag="u1s")
            u2s = cpool.tile([128, 2, UCOLS], FP, tag="u2s")
            vout = vpool.tile([128, VCOLS], FP, tag="v")

            # fixed PSUM regions: [m-half, tile-parity, row] = m*1024 + t*512
            ps1 = pspool.tile([128, 2 * TILE * 2], FP, tag="ps1", name="ps1")
            ps2 = pspool.tile([128, 2 * TILE * 2], FP, tag="ps2", name="ps2")

            h0tiles = {}

            def h0tile(g):
                if g not in h0tiles:
                    h0tiles[g] = hpool.tile(
                        [128, 2, GROUP_ROWS], FP, tag="h0g", name=f"h0g{g}"
                    )
                return h0tiles[g]

            t1tiles = {}

            def t1tile(w):
                if w not in t1tiles:
                    t1tiles[w] = t1pool.tile(
                        [128, 2, WINROWS], FP, tag="t1w", name=f"t1w{w}"
                    )
                return t1tiles[w]

            t2tiles = {}

            def t2tile(w):
                if w not in t2tiles:
                    t2tiles[w] = t2pool.tile(
                        [128, 2, WINROWS], FP, tag="t2w", name=f"t2w{w}"
                    )
                return t2tiles[w]

            def brd_piece(g, k, eng=None):
                """One broadcast-add (h=k//4, quad=k%4) of group g's h0.

                Steady-state pieces run on the (otherwise idle) GpSimd so
                the VectorE queue stays short for the latency-critical
                resid -> L2 edge; group 0 runs on VectorE (startup latency).
                """
                h, q = divmod(k, 4)
                a, b2 = _QUADS[q]
                u1h = u1s[:, h, :].rearrange("p (w i) -> p w i", i=NPERM)
                u2h = u2s[:, h, :].rearrange("p (w j) -> p w j", j=NPERM)
                outh = h0tile(g)[:, h, :].rearrange(
                    "p (w r) -> p w r", r=NPAIR
                )
                w0 = g * GROUP_WALKERS
                w1_ = w0 + GROUP_WALKERS
                out_ap = outh[:, :, q * QUAD:(q + 1) * QUAD].rearrange(
                    "p w (i j) -> p w i j", j=12
                )
                in1 = u1h[:, w0:w1_, a * 12:(a + 1) * 12].rearrange(
                    "p w (i u) -> p w i u", u=1
                ).broadcast_to([128, GROUP_WALKERS, 12, 12])
                in2 = u2h[:, w0:w1_, b2 * 12:(b2 + 1) * 12].rearrange(
                    "p w (u j) -> p w u j", u=1
                ).broadcast_to([128, GROUP_WALKERS, 12, 12])
                (eng or nc.vector).tensor_add(out_ap, in1, in2)

            def tanh0_chunk(g, k, nk=3):
                """tanh0 chunk k (of nk) of group g, in place in the ring."""
                c0 = k * (GROUP_ROWS // nk)
                c1 = (k + 1) * (GROUP_ROWS // nk)
                ap = h0tile(g)[:, :, c0:c1]
                nc.scalar.activation(ap, ap, TANH)

            def l1_mm(j):
                """Layer-1 matmuls for tile j into ps1."""
                g, sg = divmod(j, TILES_PER_GROUP)
                t = j % 2
                h0g = h0tile(g)
                for m in (0, 1):
                    dst = ps1[:, m * 2 * TILE + t * TILE:
                              m * 2 * TILE + (t + 1) * TILE]
                    for c, wsb in ((0, w1a), (1, w1b)):
                        nc.tensor.matmul(
                            dst,
                            wsb[:, m * 128:(m + 1) * 128],
                            h0g[:, c, sg * TILE:(sg + 1) * TILE],
                            start=(c == 0),
                            stop=(c == 1),
                        )

            def tanh1(s2):
                """tanh1 for supertile s2 (tiles 2*s2, 2*s2+1): one instr."""
                t1w = t1tile(s2 // 2)
                off = (s2 % 2) * 2 * TILE
                dst = t1w[:, :, off:off + 2 * TILE]
                src = ps1[:].rearrange("p (h r) -> p h r", h=2)
                if with_bias:
                    for m in (0, 1):
                        nc.scalar.activation(
                            dst[:, m, :],
                            ps1[:, m * 2 * TILE:(m + 1) * 2 * TILE],
                            TANH,
                            bias=bsb[:, m:m + 1],
                        )
                else:
                    nc.scalar.activation(dst, src, TANH)

            def resid(s2):
                """t1 += tanh0 for supertile s2 (merged add when the two
                tiles share an h0 group, split at group crossings)."""
                j0 = 2 * s2
                t1w = t1tile(s2 // 2)
                g0, sg0 = divmod(j0, TILES_PER_GROUP)
                g1, sg1 = divmod(j0 + 1, TILES_PER_GROUP)
                off = (j0 % 4) * TILE
                if g0 == g1:
                    sl = t1w[:, :, off:off + 2 * TILE]
                    nc.gpsimd.tensor_add(
                        sl, sl,
                        h0tile(g0)[:, :, sg0 * TILE:(sg0 + 2) * TILE],
                    )
                else:
                    for g, sg, o in ((g0, sg0, off), (g1, sg1, off + TILE)):
                        sl = t1w[:, :, o:o + TILE]
                        nc.gpsimd.tensor_add(
                            sl, sl,
                            h0tile(g)[:, :, sg * TILE:(sg + 1) * TILE],
                        )

            def l2_mm(s2):
                """Layer-2 matmuls for supertile s2 into ps2."""
                t1w = t1tile(s2 // 2)
                off = (s2 % 2) * 2 * TILE
                for t in (0, 1):
                    mov_off = off + t * TILE
                    for m in (0, 1):
                        dst = ps2[:, m * 2 * TILE + t * TILE:
                                  m * 2 * TILE + (t + 1) * TILE]
                        for c, wsb in ((0, w2a), (1, w2b)):
                            nc.tensor.matmul(
                                dst,
                                wsb[:, m * 128:(m + 1) * 128],
                                t1w[:, c, mov_off:mov_off + TILE],
                                start=(c == 0),
                                stop=(c == 1),
                            )

            def tanh2(s2):
                """tanh2 for supertile s2: one instr into the t2 window."""
                t2w = t2tile(s2 // 2)
                off = (s2 % 2) * 2 * TILE
                dst = t2w[:, :, off:off + 2 * TILE]
                src = ps2[:].rearrange("p (h r) -> p h r", h=2)
                if with_bias:
                    for m in (0, 1):
                        nc.scalar.activation(
                            dst[:, m, :],
                            ps2[:, m * 2 * TILE:(m + 1) * 2 * TILE],
                            TANH,
                            bias=bsb[:, 2 + m:3 + m],
                        )
                else:
                    nc.scalar.activation(dst, src, TANH)

            def red(w, l):
                """Block sums of window w, tensor l (0: t1, 1: t2)."""
                src = (t1tile(w) if l == 0 else t2tile(w))[:].rearrange(
                    "p h (b e) -> p h b e", e=BLK
                )
                base = w * WCOLS + l * 2 * BPW
                dst = vout[:, base:base + 2 * BPW].rearrange(
                    "p (h b) -> p h b", h=2
                )
                nc.vector.reduce_sum(dst, src, axis=AXX)

            # ---- u1s/u2s: first-layer partials, columns (walker, sorted perm)
            # c1/c2 go to ps2: their drain copies queue behind the group-0
            # bcasts on the VectorE, and ps1's first main-loop writer L1(0)
            # must not wait on them (ps2's first writer L2(0) runs much later)
            for c in range(UCOLS // TILE):
                psu = (ps1, ps2, ps2)[c]
                for ui, (usb, uf, uw) in enumerate(
                    ((u1s, x1f, x1w), (u2s, x2f, x2w))
                ):
                    for h in (0, 1):
                        nc.tensor.matmul(
                            psu[:, ui * 2 * TILE + h * TILE:
                                ui * 2 * TILE + (h + 1) * TILE],
                            uw[:, h * 128:(h + 1) * 128],
                            uf[:, c * TILE:(c + 1) * TILE],
                            start=True,
                            stop=True,
                        )
                    nc.vector.tensor_copy(
                        usb[:, :, c * TILE:(c + 1) * TILE],
                        psu[:, ui * 2 * TILE:(ui + 1) * 2 * TILE].rearrange(
                            "p (h r) -> p h r", h=2
                        ),
                    )
                if c == 0:
                    for k in range(8):
                        brd_piece(0, k, eng=nc.vector)
                    # chunked for group 0 so L1(tile 0) waits on 1/3 only
                    for k in range(3):
                        tanh0_chunk(0, k)

            # ---- main loop over psum supertiles
            def spread(j):
                """Next-group h0 construction, spread across this group.
                The resid adds live on GpSimd, so the bcast adds can't
                delay the latency-critical tanh1 -> resid -> L2 edge."""
                g, sg = divmod(j, TILES_PER_GROUP)
                if g + 1 >= NGROUPS:
                    return
                if sg <= 1:
                    for k in range(4):
                        brd_piece(g + 1, 4 * sg + k)
                elif 2 <= sg <= 4:
                    tanh0_chunk(g + 1, sg - 2)

            # staged vout drains: window w's columns are final once
            # red(w, 1) has issued (at s2 = 2w + 3)
            drains = {15: (0, 6), 25: (6, 11), 33: (11, 15)}

            for s2 in range(NSUPER):
                l1_mm(2 * s2)
                spread(2 * s2)
                l1_mm(2 * s2 + 1)
                spread(2 * s2 + 1)
                tanh1(s2)
                resid(s2)
                if s2 > 0:
                    l2_mm(s2 - 1)
                    tanh2(s2 - 1)
                # window w complete (tanh2(2w+1) issued) at s2 = 2w+2;
                # stagger t2's reduce one super later so it lands in slack
                if s2 >= 2 and s2 % 2 == 0:
                    red((s2 - 2) // 2, 0)
                if s2 >= 3 and s2 % 2 == 1:
                    red((s2 - 3) // 2, 1)
                if s2 in drains:
                    w0, w1_ = drains[s2]
                    nc.sync.dma_start(
                        v_d[:, w0 * WCOLS:w1_ * WCOLS],
                        vout[:, w0 * WCOLS:w1_ * WCOLS],
                    )

            l2_mm(NSUPER - 1)
            tanh2(NSUPER - 1)
            red(NWIN - 1, 0)
            red(NWIN - 1, 1)
            nc.sync.dma_start(
                v_d[:, 15 * WCOLS:], vout[:, 15 * WCOLS:]
            )

    nc.compile()
    _cached[key] = nc
    return nc


def _build_feats(x1, x2):
    """Per-walker first-layer inputs in sign-sorted perm order.

    Returns (X1f [B, 24, 13], X2f [B, 24, 12]): X1f[b, ip] = flattened
    x1[b, P1[_ISORT[ip]]] + trailing 1.0 (carries b0); X2f likewise, no ones.
    """
    B = x1.shape[0]
    xp1 = x1[:, _P1[_ISORT], :].reshape(B, NPERM, N1 * D)
    xp2 = x2[:, _P2[_JSORT], :].reshape(B, NPERM, N2 * D)
    X1f = np.empty((B, NPERM, K1), dtype=np.float32)
    X1f[:, :, :N1 * D] = xp1
    X1f[:, :, N1 * D] = 1.0
    return X1f, np.ascontiguousarray(xp2)


def _make_in_maps(x1, x2, W0, b0, W1, b1, W2, b2):
    with_bias = bool(np.any(b1) or np.any(b2))
    X1f, X2f = _build_feats(x1, x2)
    x1w = np.concatenate([W0[:N1 * D], b0[None, :]], axis=0)  # [13, 256]
    x2w = W0[N1 * D:]  # [12, 256]
    xww = _pack32(x1w, x2w)  # [44, 256]: x1 rows at 0, x2 rows at 32
    wpk = np.ascontiguousarray(np.concatenate(
        [W1[0:128], W1[128:256], W2[0:128], W2[128:256]], axis=1
    ))  # [128, 1024]
    in_maps = []
    for c in range(NCORES):
        sl = slice(c * WALKERS_PER_CORE, (c + 1) * WALKERS_PER_CORE)
        m = {
            "xf": _pack32(X1f[sl].reshape(UCOLS, K1).T,
                          X2f[sl].reshape(UCOLS, K2).T),
            "xw": xww,
            "w": wpk,
        }
        if with_bias:
            bm = np.zeros((128, 4), dtype=np.float32)
            bm[:, 0] = b1[0:128]
            bm[:, 1] = b1[128:256]
            bm[:, 2] = b2[0:128]
            bm[:, 3] = b2[128:256]
            m["b12"] = bm
        in_maps.append(m)
    return with_bias, in_maps


def _pack32(a, b):
    """Stack a (<=32 rows) at partition 0 and b at partition 32."""
    out = np.zeros((32 + b.shape[0], a.shape[1]), dtype=np.float32)
    out[:a.shape[0]] = a
    out[32:] = b
    return out


# static host-side combine map: vout column -> (walker, sign) segment
def _make_combine_map():
    # columns viewed as [w (18), l (2), h (2), b (64)]; block row0 =
    # w*WINROWS + b*BLK; 32-row blocks never straddle walker (576) or
    # sign (288) boundaries.
    w_idx = np.arange(NWIN)[:, None]
    b_idx = np.arange(BPW)[None, :]
    row0 = w_idx * WINROWS + b_idx * BLK          # [18, 64]
    walker = row0 // NPAIR
    sign = (row0 % NPAIR) // HALFPAIR
    seg = (walker * 2 + sign).reshape(-1)          # [1152] per (w, b)
    order = np.argsort(seg, kind="stable")
    starts = np.searchsorted(seg[order], np.arange(2 * WALKERS_PER_CORE))
    return order, starts


_ORDER, _STARTS = _make_combine_map()


def _combine_core(v):
    """vout [128, 4608] -> per-walker signed sums [256, 64] in fp64."""
    v5 = v.reshape(128, NWIN, 2, 2, BPW).astype(np.float64)
    vt = v5.sum(axis=2)                      # [p, w, h, b]  (t1 + t2)
    vt = vt.transpose(2, 0, 1, 3).reshape(2, 128, NWIN * BPW)  # [h, p, wb]
    vs = np.add.reduceat(vt[:, :, _ORDER], _STARTS, axis=2)
    # vs: [h, p, walker*2 + sign]
    d = vs[:, :, 0::2] - vs[:, :, 1::2]      # [h, p, walker]
    return d.reshape(NDENSE, WALKERS_PER_CORE)


def _finish(v_per_core, Wf, bf):
    """per-core vout -> log|anti| [BATCH]."""
    out = np.empty((BATCH,), dtype=np.float32)
    wf64 = Wf[:, 0].astype(np.float64)
    # sum of pair signs is exactly 0, so bf drops out of the signed sum
    for c in range(NCORES):
        u = _combine_core(v_per_core[c])
        anti = wf64 @ u
        out[c * WALKERS_PER_CORE:(c + 1) * WALKERS_PER_CORE] = np.log(
            np.abs(anti)
        ).astype(np.float32)
    return out


def kernel(x1, x2, W0, b0, W1, b1, W2, b2, Wf, bf):
    from concourse.bass_utils import run_bass_kernel_spmd

    x1 = np.asarray(x1, dtype=np.float32)
    x2 = np.asarray(x2, dtype=np.float32)
    W0 = np.asarray(W0, dtype=np.float32)
    b0 = np.asarray(b0, dtype=np.float32)
    W1 = np.asarray(W1, dtype=np.float32)
    b1 = np.asarray(b1, dtype=np.float32)
    W2 = np.asarray(W2, dtype=np.float32)
    b2 = np.asarray(b2, dtype=np.float32)
    Wf = np.asarray(Wf, dtype=np.float32)
    bf = np.asarray(bf, dtype=np.float32)

    with_bias, in_maps = _make_in_maps(x1, x2, W0, b0, W1, b1, W2, b2)
    nc = _build_nc(with_bias)

    res = run_bass_kernel_spmd(nc, in_maps, list(range(NCORES)))
    global _last_results
    _last_results = res

    return _finish([res.results[c]["v"] for c in range(NCORES)], Wf, bf)


# revision 20
# speedup vs baseline: 1.0105x; 1.0105x over previous
"""Trainium2 Bass kernel for the brute-force antisymmetrized ResNet.

Math (per walker b):
    feats[i,j] = concat(x1[P1[i]], x2[P2[j]]).reshape(24)    (576 = 24*24 perm pairs)
    y0 = tanh(feats @ W0 + b0)
    y1 = tanh(y0 @ W1 + b1) + y0
    y2 = tanh(y1 @ W2 + b2) + y1
    out[b] = log| sum_{i,j} s1[i] s2[j] (y2 @ Wf + bf) |

Strategy:
  - Data-parallel over the 512 walkers: 64 walkers per NeuronCore x 8 cores.
  - First layer is factored: y0pre[b,i,j] = u1[b,i] + u2[b,j] where
    u1[b,i] = x1feats(perm i) @ W0[:12] + b0 and u2[b,j] = x2feats(perm j) @
    W0[12:24].  The device computes u1/u2 with tiny matmuls and
    broadcast-adds them into the 576 rows per walker on VectorE.
  - Permutations are sign-sorted into quadrants so each walker's 576 rows are
    ordered [(+,+), (-,-), (+,-), (-,+)] x 12 x 12: the first 288 rows have
    pair-sign +1, the last 288 have -1.
  - Plain fp32 matmuls (fp32r/tf32 rounding is amplified catastrophically by
    the antisymmetrization: |anti| reaches 2.6e-4 against O(1) terms).
  - Instruction-count-minimized dataflow (~1065 instructions vs ~1980 for
    the per-piece predecessor): fixed 4-bank PSUM regions (ps1 for layer 1,
    ps2 for layer 2) let tanh1/tanh2 each cover TWO 512-row tiles in a
    single ScalarE instruction; tanh0 is 3 instructions per 4608-row group
    (in the h0 ring); inputs arrive in 3 packed DMAs.
  - The sign-segment reduction is done as per-32-row block sums
    (32 = gcd(512, 288), so blocks never straddle walker/sign boundaries)
    with ONE 4D-AP VectorE reduce per 2048-row window per tensor; the host
    combines block sums in fp64 and applies Wf and log|.|.  The short fp32
    accumulation chains cut max rel err ~6x vs 288-row device sums.
  - sum(y2) = sum(t1) + sum(tanh2) per block (y2 = tanh2 + t1), so only
    those two tensors are reduced on device.
  - Engine split: PE matmuls; ScalarE tanh; VectorE broadcast-adds +
    block reduces; GpSimd the t1 residual adds, so the latency-critical
    tanh1 -> resid -> layer-2 edge never queues behind VectorE work.
    The next group's h0 bcasts/tanh0 are spread over tiles sg 0..4 so the
    chain completes well before the group boundary.
"""

import itertools

import numpy as np

N1 = 4
N2 = 4
D = 3
BATCH = 512
NDENSE = 256
NCORES = 8
NPERM = 24              # 4!
NPAIR = NPERM * NPERM   # 576
HALFPAIR = NPAIR // 2   # 288 rows of each sign per walker
QUAD = NPERM // 2 * (NPERM // 2)            # 144 rows per quadrant
WALKERS_PER_CORE = BATCH // NCORES          # 64
ROWS_PER_CORE = WALKERS_PER_CORE * NPAIR    # 36864
TILE = 512                                  # matmul moving-dim tile
GROUP_WALKERS = 8                           # walkers per h0-ring group
GROUP_ROWS = GROUP_WALKERS * NPAIR          # 4608 = 9 * TILE
TILES_PER_GROUP = GROUP_ROWS // TILE        # 9
NGROUPS = ROWS_PER_CORE // GROUP_ROWS       # 8
NTILES = NGROUPS * TILES_PER_GROUP          # 72
NSUPER = NTILES // 2                        # 36 psum supertiles (1024 rows)
NWIN = NTILES // 4                          # 18 reduce windows (2048 rows)
WINROWS = 4 * TILE                          # 2048
BLK = 32                                    # gcd(TILE, HALFPAIR)
BPW = WINROWS // BLK                        # 64 blocks per window
WCOLS = 2 * 2 * BPW                         # vout cols per window: l, h, b
VCOLS = NWIN * WCOLS                        # 4608
UCOLS = WALKERS_PER_CORE * NPERM            # 1536 u-columns per core
K1 = N1 * D + 1                             # 13: x1 features + ones row (b0)
K2 = N2 * D                                 # 12


def _perms_and_signs(n):
    P = np.array(list(itertools.permutations(range(n))), dtype=np.int32)
    triu = np.triu(np.ones((n, n), dtype=np.int64), 1)
    inv = np.sum((P[:, :, None] > P[:, None, :]) * triu, axis=(1, 2))
    signs = np.where(inv % 2 == 0, 1.0, -1.0).astype(np.float32)
    return P, signs


_P1, _S1 = _perms_and_signs(N1)
_P2, _S2 = _perms_and_signs(N2)

# sign-sorted perm orders: 12 even perms then 12 odd perms
_ISORT = np.concatenate([np.where(_S1 > 0)[0], np.where(_S1 < 0)[0]])
_JSORT = np.concatenate([np.where(_S2 > 0)[0], np.where(_S2 < 0)[0]])
# quadrants (a, b): pair-sign = +1 for the first two, -1 for the last two
_QUADS = [(0, 0), (1, 1), (0, 1), (1, 0)]

_cached = {}
_last_results = None  # BassKernelResults of the most recent run (for profiling)


def _build_nc(with_bias: bool):
    """Build + compile the 8-core SPMD Tile kernel (cached)."""
    key = bool(with_bias)
    if key in _cached:
        return _cached[key]

    import concourse.bacc as bacc
    import concourse.tile as tile
    from concourse import mybir

    FP = mybir.dt.float32
    TANH = mybir.ActivationFunctionType.Tanh
    AXX = mybir.AxisListType.X

    nc = bacc.Bacc(
        "TRN2",
        target_bir_lowering=False,
        debug=False,
        num_devices=NCORES,
    )

    # packed inputs: one DMA each for perm-features, first-layer weights,
    # and the four 128-row halves of W1/W2
    # x2 rows sit at partition 32 (matmul base partition must be 0/32/64)
    xf_d = nc.dram_tensor("xf", [32 + K2, UCOLS], FP, kind="ExternalInput").ap()
    xw_d = nc.dram_tensor("xw", [32 + K2, NDENSE], FP, kind="ExternalInput").ap()
    w_d = nc.dram_tensor("w", [128, 4 * NDENSE], FP, kind="ExternalInput").ap()
    if with_bias:
        b_d = nc.dram_tensor("b12", [128, 4], FP, kind="ExternalInput").ap()
    v_d = nc.dram_tensor("v", [128, VCOLS], FP, kind="ExternalOutput").ap()

    with tile.TileContext(nc) as tc:
        with (
            tc.tile_pool(name="consts", bufs=1) as cpool,
            tc.tile_pool(name="t1p", bufs=2) as t1pool,
            tc.tile_pool(name="t2p", bufs=2) as t2pool,
            tc.tile_pool(name="h0ring", bufs=2) as hpool,
            tc.tile_pool(name="vout", bufs=1) as vpool,
            tc.tile_pool(name="ps", bufs=1, space="PSUM") as pspool,
        ):
            xf = cpool.tile([32 + K2, UCOLS], FP, tag="xf")
            nc.sync.dma_start(xf[:], xf_d[:])
            xw = cpool.tile([32 + K2, NDENSE], FP, tag="xw")
            nc.sync.dma_start(xw[:], xw_d[:])
            wsb = cpool.tile([128, 4 * NDENSE], FP, tag="wsb")
            nc.sync.dma_start(wsb[:], w_d[:])
            x1f, x2f = xf[0:K1, :], xf[32:32 + K2, :]
            x1w, x2w = xw[0:K1, :], xw[32:32 + K2, :]
            w1a = wsb[:, 0 * NDENSE:1 * NDENSE]
            w1b = wsb[:, 1 * NDENSE:2 * NDENSE]
            w2a = wsb[:, 2 * NDENSE:3 * NDENSE]
            w2b = wsb[:, 3 * NDENSE:4 * NDENSE]
            if with_bias:
                bsb = cpool.tile([128, 4], FP, tag="b12")  # b1h0 b1h1 b2h0 b2h1
                nc.sync.dma_start(bsb[:], b_d[:])

            u1s = cpool.tile([128, 2, UCOLS], FP, tag="u1s")
            u2s = cpool.tile([128, 2, UCOLS], FP, tag="u2s")
            vout = vpool.tile([128, VCOLS], FP, tag="v")

            # fixed PSUM regions: [m-half, tile-parity, row] = m*1024 + t*512
            ps1 = pspool.tile([128, 2 * TILE * 2], FP, tag="ps1", name="ps1")
            ps2 = pspool.tile([128, 2 * TILE * 2], FP, tag="ps2", name="ps2")

            h0tiles = {}

            def h0tile(g):
                if g not in h0tiles:
                    h0tiles[g] = hpool.tile(
                        [128, 2, GROUP_ROWS], FP, tag="h0g", name=f"h0g{g}"
                    )
                return h0tiles[g]

            t1tiles = {}

            def t1tile(w):
                if w not in t1tiles:
                    t1tiles[w] = t1pool.tile(
                        [128, 2, WINROWS], FP, tag="t1w", name=f"t1w{w}"
                    )
                return t1tiles[w]

            t2tiles = {}

            def t2tile(w):
                if w not in t2tiles:
                    t2tiles[w] = t2pool.tile(
                        [128, 2, WINROWS], FP, tag="t2w", name=f"t2w{w}"
                    )
                return t2tiles[w]

            def brd_piece(g, k, eng=None):
                """One broadcast-add (h=k//4, quad=k%4) of group g's h0.

                Steady-state pieces run on the (otherwise idle) GpSimd so
                the VectorE queue stays short for the latency-critical
                resid -> L2 edge; group 0 runs on VectorE (startup latency).
                """
                h, q = divmod(k, 4)
                a, b2 = _QUADS[q]
                u1h = u1s[:, h, :].rearrange("p (w i) -> p w i", i=NPERM)
                u2h = u2s[:, h, :].rearrange("p (w j) -> p w j", j=NPERM)
                outh = h0tile(g)[:, h, :].rearrange(
                    "p (w r) -> p w r", r=NPAIR
                )
                w0 = g * GROUP_WALKERS
                w1_ = w0 + GROUP_WALKERS
                out_ap = outh[:, :, q * QUAD:(q + 1) * QUAD].rearrange(
                    "p w (i j) -> p w i j", j=12
                )
                in1 = u1h[:, w0:w1_, a * 12:(a + 1) * 12].rearrange(
                    "p w (i u) -> p w i u", u=1
                ).broadcast_to([128, GROUP_WALKERS, 12, 12])
                in2 = u2h[:, w0:w1_, b2 * 12:(b2 + 1) * 12].rearrange(
                    "p w (u j) -> p w u j", u=1
                ).broadcast_to([128, GROUP_WALKERS, 12, 12])
                (eng or nc.vector).tensor_add(out_ap, in1, in2)

            def tanh0_chunk(g, k, nk=3, h=None):
                """tanh0 chunk k (of nk) of group g, in place in the ring.
                h selects one channel half (group-0 startup: the c=0 layer-1
                matmuls only need the h=0 half)."""
                c0 = k * (GROUP_ROWS // nk)
                c1 = (k + 1) * (GROUP_ROWS // nk)
                hs = slice(None) if h is None else slice(h, h + 1)
                ap = h0tile(g)[:, hs, c0:c1]
                nc.scalar.activation(ap, ap, TANH)

            def l1_mm(j):
                """Layer-1 matmuls for tile j into ps1."""
                g, sg = divmod(j, TILES_PER_GROUP)
                t = j % 2
                h0g = h0tile(g)
                for m in (0, 1):
                    dst = ps1[:, m * 2 * TILE + t * TILE:
                              m * 2 * TILE + (t + 1) * TILE]
                    for c, wsb in ((0, w1a), (1, w1b)):
                        nc.tensor.matmul(
                            dst,
                            wsb[:, m * 128:(m + 1) * 128],
                            h0g[:, c, sg * TILE:(sg + 1) * TILE],
                            start=(c == 0),
                            stop=(c == 1),
                        )

            def tanh1(s2):
                """tanh1 for supertile s2 (tiles 2*s2, 2*s2+1): one instr."""
                t1w = t1tile(s2 // 2)
                off = (s2 % 2) * 2 * TILE
                dst = t1w[:, :, off:off + 2 * TILE]
                src = ps1[:].rearrange("p (h r) -> p h r", h=2)
                if with_bias:
                    for m in (0, 1):
                        nc.scalar.activation(
                            dst[:, m, :],
                            ps1[:, m * 2 * TILE:(m + 1) * 2 * TILE],
                            TANH,
                            bias=bsb[:, m:m + 1],
                        )
                else:
                    nc.scalar.activation(dst, src, TANH)

            def resid(s2):
                """t1 += tanh0 for supertile s2 (merged add when the two
                tiles share an h0 group, split at group crossings)."""
                j0 = 2 * s2
                t1w = t1tile(s2 // 2)
                g0, sg0 = divmod(j0, TILES_PER_GROUP)
                g1, sg1 = divmod(j0 + 1, TILES_PER_GROUP)
                off = (j0 % 4) * TILE
                if g0 == g1:
                    sl = t1w[:, :, off:off + 2 * TILE]
                    nc.gpsimd.tensor_add(
                        sl, sl,
                        h0tile(g0)[:, :, sg0 * TILE:(sg0 + 2) * TILE],
                    )
                else:
                    for g, sg, o in ((g0, sg0, off), (g1, sg1, off + TILE)):
                        sl = t1w[:, :, o:o + TILE]
                        nc.gpsimd.tensor_add(
                            sl, sl,
                            h0tile(g)[:, :, sg * TILE:(sg + 1) * TILE],
                        )

            def l2_mm(s2):
                """Layer-2 matmuls for supertile s2 into ps2."""
                t1w = t1tile(s2 // 2)
                off = (s2 % 2) * 2 * TILE
                for t in (0, 1):
                    mov_off = off + t * TILE
                    for m in (0, 1):
                        dst = ps2[:, m * 2 * TILE + t * TILE:
                                  m * 2 * TILE + (t + 1) * TILE]
                        for c, wsb in ((0, w2a), (1, w2b)):
                            nc.tensor.matmul(
                                dst,
                                wsb[:, m * 128:(m + 1) * 128],
                                t1w[:, c, mov_off:mov_off + TILE],
                                start=(c == 0),
                                stop=(c == 1),
                            )

            def tanh2(s2):
                """tanh2 for supertile s2: one instr into the t2 window."""
                t2w = t2tile(s2 // 2)
                off = (s2 % 2) * 2 * TILE
                dst = t2w[:, :, off:off + 2 * TILE]
                src = ps2[:].rearrange("p (h r) -> p h r", h=2)
                if with_bias:
                    for m in (0, 1):
                        nc.scalar.activation(
                            dst[:, m, :],
                            ps2[:, m * 2 * TILE:(m + 1) * 2 * TILE],
                            TANH,
                            bias=bsb[:, 2 + m:3 + m],
                        )
                else:
                    nc.scalar.activation(dst, src, TANH)

            def red(w, l, b0=0, b1=BPW):
                """Block sums [b0:b1] of window w, tensor l (0: t1, 1: t2)."""
                src = (t1tile(w) if l == 0 else t2tile(w))[:].rearrange(
                    "p h (b e) -> p h b e", e=BLK
                )[:, :, b0:b1, :]
                base = w * WCOLS + l * 2 * BPW
                dst = vout[:, base:base + 2 * BPW].rearrange(
                    "p (h b) -> p h b", h=2
                )[:, :, b0:b1]
                nc.vector.reduce_sum(dst, src, axis=AXX)

            # ---- u1s/u2s: first-layer partials, columns (walker, sorted perm)
            # c1/c2 go to ps2: their drain copies queue behind the group-0
            # bcasts on the VectorE, and ps1's first main-loop writer L1(0)
            # must not wait on them (ps2's first writer L2(0) runs much later)
            for c in range(UCOLS // TILE):
                psu = (ps1, ps2, ps2)[c]
                for ui, (usb, uf, uw) in enumerate(
                    ((u1s, x1f, x1w), (u2s, x2f, x2w))
                ):
                    for h in (0, 1):
                        nc.tensor.matmul(
                            psu[:, ui * 2 * TILE + h * TILE:
                                ui * 2 * TILE + (h + 1) * TILE],
                            uw[:, h * 128:(h + 1) * 128],
                            uf[:, c * TILE:(c + 1) * TILE],
                            start=True,
                            stop=True,
                        )
                    nc.vector.tensor_copy(
                        usb[:, :, c * TILE:(c + 1) * TILE],
                        psu[:, ui * 2 * TILE:(ui + 1) * 2 * TILE].rearrange(
                            "p (h r) -> p h r", h=2
                        ),
                    )
                if c == 0:
                    # h=0 pieces + h=0 tanh halves first: the c=0 layer-1
                    # matmuls of the first tiles depend only on these
                    for k in range(4):
                        brd_piece(0, k, eng=nc.vector)
                    for k in range(3):
                        tanh0_chunk(0, k, h=0)
                    for k in range(4, 8):
                        brd_piece(0, k, eng=nc.vector)
                    for k in range(3):
                        tanh0_chunk(0, k, h=1)

            # ---- main loop over psum supertiles
            def spread(j):
                """Next-group h0 construction, spread across this group.
                The resid adds live on GpSimd, so the bcast adds can't
                delay the latency-critical tanh1 -> resid -> L2 edge."""
                g, sg = divmod(j, TILES_PER_GROUP)
                if g + 1 >= NGROUPS:
                    return
                if sg <= 1:
                    for k in range(4):
                        brd_piece(g + 1, 4 * sg + k)
                elif 2 <= sg <= 4:
                    tanh0_chunk(g + 1, sg - 2)

            # staged vout drains: window w's columns are final once
            # red(w, 1) has issued (at s2 = 2w + 3)
            drains = {15: (0, 6), 25: (6, 11), 33: (11, 16), 35: (16, 17)}

            for s2 in range(NSUPER):
                l1_mm(2 * s2)
                spread(2 * s2)
                l1_mm(2 * s2 + 1)
                spread(2 * s2 + 1)
                tanh1(s2)
                resid(s2)
                if s2 > 0:
                    l2_mm(s2 - 1)
                    tanh2(s2 - 1)
                # window w complete (tanh2(2w+1) issued) at s2 = 2w+2;
                # stagger t2's reduce one super later so it lands in slack
                if s2 >= 2 and s2 % 2 == 0:
                    red((s2 - 2) // 2, 0)
                if s2 >= 3 and s2 % 2 == 1:
                    red((s2 - 3) // 2, 1)
                if s2 == NSUPER - 1:
                    # first half of the last window is already final: its
                    # tanh1/resid (super 34) and tanh2(34) are in
                    red(NWIN - 1, 0, 0, BPW // 2)
                    red(NWIN - 1, 1, 0, BPW // 2)
                if s2 in drains:
                    w0, w1_ = drains[s2]
                    nc.sync.dma_start(
                        v_d[:, w0 * WCOLS:w1_ * WCOLS],
                        vout[:, w0 * WCOLS:w1_ * WCOLS],
                    )

            l2_mm(NSUPER - 1)
            tanh2(NSUPER - 1)
            red(NWIN - 1, 0, BPW // 2, BPW)
            red(NWIN - 1, 1, BPW // 2, BPW)
            nc.sync.dma_start(
                v_d[:, 17 * WCOLS:], vout[:, 17 * WCOLS:]
            )

    nc.compile()
    _cached[key] = nc
    return nc


def _build_feats(x1, x2):
    """Per-walker first-layer inputs in sign-sorted perm order.

    Returns (X1f [B, 24, 13], X2f [B, 24, 12]): X1f[b, ip] = flattened
    x1[b, P1[_ISORT[ip]]] + trailing 1.0 (carries b0); X2f likewise, no ones.
    """
    B = x1.shape[0]
    xp1 = x1[:, _P1[_ISORT], :].reshape(B, NPERM, N1 * D)
    xp2 = x2[:, _P2[_JSORT], :].reshape(B, NPERM, N2 * D)
    X1f = np.empty((B, NPERM, K1), dtype=np.float32)
    X1f[:, :, :N1 * D] = xp1
    X1f[:, :, N1 * D] = 1.0
    return X1f, np.ascontiguousarray(xp2)


def _make_in_maps(x1, x2, W0, b0, W1, b1, W2, b2):
    with_bias = bool(np.any(b1) or np.any(b2))
    X1f, X2f = _build_feats(x1, x2)
    x1w = np.concatenate([W0[:N1 * D], b0[None, :]], axis=0)  # [13, 256]
    x2w = W0[N1 * D:]  # [12, 256]
    xww = _pack32(x1w, x2w)  # [44, 256]: x1 rows at 0, x2 rows at 32
    wpk = np.ascontiguousarray(np.concatenate(
        [W1[0:128], W1[128:256], W2[0:128], W2[128:256]], axis=1
    ))  # [128, 1024]
    in_maps = []
    for c in range(NCORES):
        sl = slice(c * WALKERS_PER_CORE, (c + 1) * WALKERS_PER_CORE)
        m = {
            "xf": _pack32(X1f[sl].reshape(UCOLS, K1).T,
                          X2f[sl].reshape(UCOLS, K2).T),
            "xw": xww,
            "w": wpk,
        }
        if with_bias:
            bm = np.zeros((128, 4), dtype=np.float32)
            bm[:, 0] = b1[0:128]
            bm[:, 1] = b1[128:256]
            bm[:, 2] = b2[0:128]
            bm[:, 3] = b2[128:256]
            m["b12"] = bm
        in_maps.append(m)
    return with_bias, in_maps


def _pack32(a, b):
    """Stack a (<=32 rows) at partition 0 and b at partition 32."""
    out = np.zeros((32 + b.shape[0], a.shape[1]), dtype=np.float32)
    out[:a.shape[0]] = a
    out[32:] = b
    return out


# static host-side combine map: vout column -> (walker, sign) segment
def _make_combine_map():
    # columns viewed as [w (18), l (2), h (2), b (64)]; block row0 =
    # w*WINROWS + b*BLK; 32-row blocks never straddle walker (576) or
    # sign (288) boundaries.
    w_idx = np.arange(NWIN)[:, None]
    b_idx = np.arange(BPW)[None, :]
    row0 = w_idx * WINROWS + b_idx * BLK          # [18, 64]
    walker = row0 // NPAIR
    sign = (row0 % NPAIR) // HALFPAIR
    seg = (walker * 2 + sign).reshape(-1)          # [1152] per (w, b)
    order = np.argsort(seg, kind="stable")
    starts = np.searchsorted(seg[order], np.arange(2 * WALKERS_PER_CORE))
    return order, starts


_ORDER, _STARTS = _make_combine_map()


def _combine_core(v):
    """vout [128, 4608] -> per-walker signed sums [256, 64] in fp64."""
    v5 = v.reshape(128, NWIN, 2, 2, BPW).astype(np.float64)
    vt = v5.sum(axis=2)                      # [p, w, h, b]  (t1 + t2)
    vt = vt.transpose(2, 0, 1, 3).reshape(2, 128, NWIN * BPW)  # [h, p, wb]
    vs = np.add.reduceat(vt[:, :, _ORDER], _STARTS, axis=2)
    # vs: [h, p, walker*2 + sign]
    d = vs[:, :, 0::2] - vs[:, :, 1::2]      # [h, p, walker]
    return d.reshape(NDENSE, WALKERS_PER_CORE)


def _finish(v_per_core, Wf, bf):
    """per-core vout -> log|anti| [BATCH]."""
    out = np.empty((BATCH,), dtype=np.float32)
    wf64 = Wf[:, 0].astype(np.float64)
    # sum of pair signs is exactly 0, so bf drops out of the signed sum
    for c in range(NCORES):
        u = _combine_core(v_per_core[c])
        anti = wf64 @ u
        out[c * WALKERS_PER_CORE:(c + 1) * WALKERS_PER_CORE] = np.log(
            np.abs(anti)
        ).astype(np.float32)
    return out


def kernel(x1, x2, W0, b0, W1, b1, W2, b2, Wf, bf):
    from concourse.bass_utils import run_bass_kernel_spmd

    x1 = np.asarray(x1, dtype=np.float32)
    x2 = np.asarray(x2, dtype=np.float32)
    W0 = np.asarray(W0, dtype=np.float32)
    b0 = np.asarray(b0, dtype=np.float32)
    W1 = np.asarray(W1, dtype=np.float32)
    b1 = np.asarray(b1, dtype=np.float32)
    W2 = np.asarray(W2, dtype=np.float32)
    b2 = np.asarray(b2, dtype=np.float32)
    Wf = np.asarray(Wf, dtype=np.float32)
    bf = np.asarray(bf, dtype=np.float32)

    with_bias, in_maps = _make_in_maps(x1, x2, W0, b0, W1, b1, W2, b2)
    nc = _build_nc(with_bias)

    res = run_bass_kernel_spmd(nc, in_maps, list(range(NCORES)))
    global _last_results
    _last_results = res

    return _finish([res.results[c]["v"] for c in range(NCORES)], Wf, bf)


# revision 21
# speedup vs baseline: 1.0105x; 1.0000x over previous
"""Trainium2 Bass kernel for the brute-force antisymmetrized ResNet.

Math (per walker b):
    feats[i,j] = concat(x1[P1[i]], x2[P2[j]]).reshape(24)    (576 = 24*24 perm pairs)
    y0 = tanh(feats @ W0 + b0)
    y1 = tanh(y0 @ W1 + b1) + y0
    y2 = tanh(y1 @ W2 + b2) + y1
    out[b] = log| sum_{i,j} s1[i] s2[j] (y2 @ Wf + bf) |

Strategy:
  - Data-parallel over the 512 walkers: 64 walkers per NeuronCore x 8 cores.
  - First layer is factored: y0pre[b,i,j] = u1[b,i] + u2[b,j] where
    u1[b,i] = x1feats(perm i) @ W0[:12] + b0 and u2[b,j] = x2feats(perm j) @
    W0[12:24].  The device computes u1/u2 with tiny matmuls and
    broadcast-adds them into the 576 rows per walker on VectorE.
  - Permutations are sign-sorted into quadrants so each walker's 576 rows are
    ordered [(+,+), (-,-), (+,-), (-,+)] x 12 x 12: the first 288 rows have
    pair-sign +1, the last 288 have -1.
  - Plain fp32 matmuls (fp32r/tf32 rounding is amplified catastrophically by
    the antisymmetrization: |anti| reaches 2.6e-4 against O(1) terms).
  - Instruction-count-minimized dataflow (~1065 instructions vs ~1980 for
    the per-piece predecessor): fixed 4-bank PSUM regions (ps1 for layer 1,
    ps2 for layer 2) let tanh1/tanh2 each cover TWO 512-row tiles in a
    single ScalarE instruction; tanh0 is 3 instructions per 4608-row group
    (in the h0 ring); inputs arrive in 3 packed DMAs.
  - The sign-segment reduction is done as per-32-row block sums
    (32 = gcd(512, 288), so blocks never straddle walker/sign boundaries)
    with ONE 4D-AP VectorE reduce per 2048-row window per tensor; the host
    combines block sums in fp64 and applies Wf and log|.|.  The short fp32
    accumulation chains cut max rel err ~6x vs 288-row device sums.
  - sum(y2) = sum(t1) + sum(tanh2) per block (y2 = tanh2 + t1), so only
    those two tensors are reduced on device.
  - Engine split: PE matmuls; ScalarE tanh; VectorE broadcast-adds +
    block reduces; GpSimd the t1 residual adds, so the latency-critical
    tanh1 -> resid -> layer-2 edge never queues behind VectorE work.
    The next group's h0 bcasts/tanh0 are spread over tiles sg 0..4 so the
    chain completes well before the group boundary.
"""

import itertools

import numpy as np

N1 = 4
N2 = 4
D = 3
BATCH = 512
NDENSE = 256
NCORES = 8
NPERM = 24              # 4!
NPAIR = NPERM * NPERM   # 576
HALFPAIR = NPAIR // 2   # 288 rows of each sign per walker
QUAD = NPERM // 2 * (NPERM // 2)            # 144 rows per quadrant
WALKERS_PER_CORE = BATCH // NCORES          # 64
ROWS_PER_CORE = WALKERS_PER_CORE * NPAIR    # 36864
TILE = 512                                  # matmul moving-dim tile
GROUP_WALKERS = 8                           # walkers per h0-ring group
GROUP_ROWS = GROUP_WALKERS * NPAIR          # 4608 = 9 * TILE
TILES_PER_GROUP = GROUP_ROWS // TILE        # 9
NGROUPS = ROWS_PER_CORE // GROUP_ROWS       # 8
NTILES = NGROUPS * TILES_PER_GROUP          # 72
NSUPER = NTILES // 2                        # 36 psum supertiles (1024 rows)
NWIN = NTILES // 4                          # 18 reduce windows (2048 rows)
WINROWS = 4 * TILE                          # 2048
BLK = 32                                    # gcd(TILE, HALFPAIR)
BPW = WINROWS // BLK                        # 64 blocks per window
WCOLS = 2 * 2 * BPW                         # vout cols per window: l, h, b
VCOLS = NWIN * WCOLS                        # 4608
UCOLS = WALKERS_PER_CORE * NPERM            # 1536 u-columns per core
K1 = N1 * D + 1                             # 13: x1 features + ones row (b0)
K2 = N2 * D                                 # 12


def _perms_and_signs(n):
    P = np.array(list(itertools.permutations(range(n))), dtype=np.int32)
    triu = np.triu(np.ones((n, n), dtype=np.int64), 1)
    inv = np.sum((P[:, :, None] > P[:, None, :]) * triu, axis=(1, 2))
    signs = np.where(inv % 2 == 0, 1.0, -1.0).astype(np.float32)
    return P, signs


_P1, _S1 = _perms_and_signs(N1)
_P2, _S2 = _perms_and_signs(N2)

# sign-sorted perm orders: 12 even perms then 12 odd perms
_ISORT = np.concatenate([np.where(_S1 > 0)[0], np.where(_S1 < 0)[0]])
_JSORT = np.concatenate([np.where(_S2 > 0)[0], np.where(_S2 < 0)[0]])
# quadrants (a, b): pair-sign = +1 for the first two, -1 for the last two
_QUADS = [(0, 0), (1, 1), (0, 1), (1, 0)]

_cached = {}
_last_results = None  # BassKernelResults of the most recent run (for profiling)


def _build_nc(with_bias: bool):
    """Build + compile the 8-core SPMD Tile kernel (cached)."""
    key = bool(with_bias)
    if key in _cached:
        return _cached[key]

    import concourse.bacc as bacc
    import concourse.tile as tile
    from concourse import mybir

    FP = mybir.dt.float32
    TANH = mybir.ActivationFunctionType.Tanh
    AXX = mybir.AxisListType.X

    nc = bacc.Bacc(
        "TRN2",
        target_bir_lowering=False,
        debug=False,
        num_devices=NCORES,
    )

    # packed inputs: one DMA each for perm-features, first-layer weights,
    # and the four 128-row halves of W1/W2
    # x2 rows sit at partition 32 (matmul base partition must be 0/32/64)
    xf_d = nc.dram_tensor("xf", [32 + K2, UCOLS], FP, kind="ExternalInput").ap()
    xw_d = nc.dram_tensor("xw", [32 + K2, NDENSE], FP, kind="ExternalInput").ap()
    w_d = nc.dram_tensor("w", [128, 4 * NDENSE], FP, kind="ExternalInput").ap()
    if with_bias:
        b_d = nc.dram_tensor("b12", [128, 4], FP, kind="ExternalInput").ap()
    v_d = nc.dram_tensor("v", [128, VCOLS], FP, kind="ExternalOutput").ap()

    with tile.TileContext(nc) as tc:
        with (
            tc.tile_pool(name="consts", bufs=1) as cpool,
            tc.tile_pool(name="t1p", bufs=2) as t1pool,
            tc.tile_pool(name="t2p", bufs=2) as t2pool,
            tc.tile_pool(name="h0ring", bufs=2) as hpool,
            tc.tile_pool(name="vout", bufs=1) as vpool,
            tc.tile_pool(name="ps", bufs=1, space="PSUM") as pspool,
        ):
            xf = cpool.tile([32 + K2, UCOLS], FP, tag="xf")
            nc.sync.dma_start(xf[:], xf_d[:])
            xw = cpool.tile([32 + K2, NDENSE], FP, tag="xw")
            nc.sync.dma_start(xw[:], xw_d[:])
            wsb = cpool.tile([128, 4 * NDENSE], FP, tag="wsb")
            nc.sync.dma_start(wsb[:], w_d[:])
            x1f, x2f = xf[0:K1, :], xf[32:32 + K2, :]
            x1w, x2w = xw[0:K1, :], xw[32:32 + K2, :]
            w1a = wsb[:, 0 * NDENSE:1 * NDENSE]
            w1b = wsb[:, 1 * NDENSE:2 * NDENSE]
            w2a = wsb[:, 2 * NDENSE:3 * NDENSE]
            w2b = wsb[:, 3 * NDENSE:4 * NDENSE]
            if with_bias:
                bsb = cpool.tile([128, 4], FP, tag="b12")  # b1h0 b1h1 b2h0 b2h1
                nc.sync.dma_start(bsb[:], b_d[:])

            u1s = cpool.tile([128, 2, UCOLS], FP, tag="u1s")
            u2s = cpool.tile([128, 2, UCOLS], FP, tag="u2s")
            vout = vpool.tile([128, VCOLS], FP, tag="v")

            # fixed PSUM regions: [m-half, tile-parity, row] = m*1024 + t*512
            ps1 = pspool.tile([128, 2 * TILE * 2], FP, tag="ps1", name="ps1")
            ps2 = pspool.tile([128, 2 * TILE * 2], FP, tag="ps2", name="ps2")

            h0tiles = {}

            def h0tile(g):
                if g not in h0tiles:
                    h0tiles[g] = hpool.tile(
                        [128, 2, GROUP_ROWS], FP, tag="h0g", name=f"h0g{g}"
                    )
                return h0tiles[g]

            t1tiles = {}

            def t1tile(w):
                if w not in t1tiles:
                    t1tiles[w] = t1pool.tile(
                        [128, 2, WINROWS], FP, tag="t1w", name=f"t1w{w}"
                    )
                return t1tiles[w]

            t2tiles = {}

            def t2tile(w):
                if w not in t2tiles:
                    t2tiles[w] = t2pool.tile(
                        [128, 2, WINROWS], FP, tag="t2w", name=f"t2w{w}"
                    )
                return t2tiles[w]

            def brd_piece(g, k, eng=None):
                """One broadcast-add (h=k//4, quad=k%4) of group g's h0.

                Steady-state pieces run on the (otherwise idle) GpSimd so
                the VectorE queue stays short for the latency-critical
                resid -> L2 edge; group 0 runs on VectorE (startup latency).
                """
                h, q = divmod(k, 4)
                a, b2 = _QUADS[q]
                u1h = u1s[:, h, :].rearrange("p (w i) -> p w i", i=NPERM)
                u2h = u2s[:, h, :].rearrange("p (w j) -> p w j", j=NPERM)
                outh = h0tile(g)[:, h, :].rearrange(
                    "p (w r) -> p w r", r=NPAIR
                )
                w0 = g * GROUP_WALKERS
                w1_ = w0 + GROUP_WALKERS
                out_ap = outh[:, :, q * QUAD:(q + 1) * QUAD].rearrange(
                    "p w (i j) -> p w i j", j=12
                )
                in1 = u1h[:, w0:w1_, a * 12:(a + 1) * 12].rearrange(
                    "p w (i u) -> p w i u", u=1
                ).broadcast_to([128, GROUP_WALKERS, 12, 12])
                in2 = u2h[:, w0:w1_, b2 * 12:(b2 + 1) * 12].rearrange(
                    "p w (u j) -> p w u j", u=1
                ).broadcast_to([128, GROUP_WALKERS, 12, 12])
                (eng or nc.vector).tensor_add(out_ap, in1, in2)

            def tanh0_chunk(g, k, nk=3, h=None):
                """tanh0 chunk k (of nk) of group g, in place in the ring.
                h selects one channel half (group-0 startup: the c=0 layer-1
                matmuls only need the h=0 half)."""
                c0 = k * (GROUP_ROWS // nk)
                c1 = (k + 1) * (GROUP_ROWS // nk)
                hs = slice(None) if h is None else slice(h, h + 1)
                ap = h0tile(g)[:, hs, c0:c1]
                nc.scalar.activation(ap, ap, TANH)

            def l1_mm(j):
                """Layer-1 matmuls for tile j into ps1."""
                g, sg = divmod(j, TILES_PER_GROUP)
                t = j % 2
                h0g = h0tile(g)
                for m in (0, 1):
                    dst = ps1[:, m * 2 * TILE + t * TILE:
                              m * 2 * TILE + (t + 1) * TILE]
                    for c, wsb in ((0, w1a), (1, w1b)):
                        nc.tensor.matmul(
                            dst,
                            wsb[:, m * 128:(m + 1) * 128],
                            h0g[:, c, sg * TILE:(sg + 1) * TILE],
                            start=(c == 0),
                            stop=(c == 1),
                        )

            def tanh1(s2):
                """tanh1 for supertile s2 (tiles 2*s2, 2*s2+1): one instr."""
                t1w = t1tile(s2 // 2)
                off = (s2 % 2) * 2 * TILE
                dst = t1w[:, :, off:off + 2 * TILE]
                src = ps1[:].rearrange("p (h r) -> p h r", h=2)
                if with_bias:
                    for m in (0, 1):
                        nc.scalar.activation(
                            dst[:, m, :],
                            ps1[:, m * 2 * TILE:(m + 1) * 2 * TILE],
                            TANH,
                            bias=bsb[:, m:m + 1],
                        )
                else:
                    nc.scalar.activation(dst, src, TANH)

            def resid(s2):
                """t1 += tanh0 for supertile s2 (merged add when the two
                tiles share an h0 group, split at group crossings)."""
                j0 = 2 * s2
                t1w = t1tile(s2 // 2)
                g0, sg0 = divmod(j0, TILES_PER_GROUP)
                g1, sg1 = divmod(j0 + 1, TILES_PER_GROUP)
                off = (j0 % 4) * TILE
                # the last super's resid is on the critical tail (no later
                # L1 work hides it): use the faster VectorE there
                eng = nc.vector if s2 >= NSUPER - 1 else nc.gpsimd
                if g0 == g1:
                    sl = t1w[:, :, off:off + 2 * TILE]
                    eng.tensor_add(
                        sl, sl,
                        h0tile(g0)[:, :, sg0 * TILE:(sg0 + 2) * TILE],
                    )
                else:
                    for g, sg, o in ((g0, sg0, off), (g1, sg1, off + TILE)):
                        sl = t1w[:, :, o:o + TILE]
                        eng.tensor_add(
                            sl, sl,
                            h0tile(g)[:, :, sg * TILE:(sg + 1) * TILE],
                        )

            def l2_mm(s2):
                """Layer-2 matmuls for supertile s2 into ps2."""
                t1w = t1tile(s2 // 2)
                off = (s2 % 2) * 2 * TILE
                for t in (0, 1):
                    mov_off = off + t * TILE
                    for m in (0, 1):
                        dst = ps2[:, m * 2 * TILE + t * TILE:
                                  m * 2 * TILE + (t + 1) * TILE]
                        for c, wsb in ((0, w2a), (1, w2b)):
                            nc.tensor.matmul(
                                dst,
                                wsb[:, m * 128:(m + 1) * 128],
                                t1w[:, c, mov_off:mov_off + TILE],
                                start=(c == 0),
                                stop=(c == 1),
                            )

            def tanh2(s2):
                """tanh2 for supertile s2: one instr into the t2 window."""
                t2w = t2tile(s2 // 2)
                off = (s2 % 2) * 2 * TILE
                dst = t2w[:, :, off:off + 2 * TILE]
                src = ps2[:].rearrange("p (h r) -> p h r", h=2)
                if with_bias:
                    for m in (0, 1):
                        nc.scalar.activation(
                            dst[:, m, :],
                            ps2[:, m * 2 * TILE:(m + 1) * 2 * TILE],
                            TANH,
                            bias=bsb[:, 2 + m:3 + m],
                        )
                else:
                    nc.scalar.activation(dst, src, TANH)

            def red(w, l, b0=0, b1=BPW):
                """Block sums [b0:b1] of window w, tensor l (0: t1, 1: t2)."""
                src = (t1tile(w) if l == 0 else t2tile(w))[:].rearrange(
                    "p h (b e) -> p h b e", e=BLK
                )[:, :, b0:b1, :]
                base = w * WCOLS + l * 2 * BPW
                dst = vout[:, base:base + 2 * BPW].rearrange(
                    "p (h b) -> p h b", h=2
                )[:, :, b0:b1]
                nc.vector.reduce_sum(dst, src, axis=AXX)

            # ---- u1s/u2s: first-layer partials, columns (walker, sorted perm)
            # c1/c2 go to ps2: their drain copies queue behind the group-0
            # bcasts on the VectorE, and ps1's first main-loop writer L1(0)
            # must not wait on them (ps2's first writer L2(0) runs much later)
            for c in range(UCOLS // TILE):
                psu = (ps1, ps2, ps2)[c]
                for ui, (usb, uf, uw) in enumerate(
                    ((u1s, x1f, x1w), (u2s, x2f, x2w))
                ):
                    for h in (0, 1):
                        nc.tensor.matmul(
                            psu[:, ui * 2 * TILE + h * TILE:
                                ui * 2 * TILE + (h + 1) * TILE],
                            uw[:, h * 128:(h + 1) * 128],
                            uf[:, c * TILE:(c + 1) * TILE],
                            start=True,
                            stop=True,
                        )
                    nc.vector.tensor_copy(
                        usb[:, :, c * TILE:(c + 1) * TILE],
                        psu[:, ui * 2 * TILE:(ui + 1) * 2 * TILE].rearrange(
                            "p (h r) -> p h r", h=2
                        ),
                    )
                if c == 0:
                    # h=0 pieces + h=0 tanh halves first: the c=0 layer-1
                    # matmuls of the first tiles depend only on these
                    for k in range(4):
                        brd_piece(0, k, eng=nc.vector)
                    for k in range(3):
                        tanh0_chunk(0, k, h=0)
                    for k in range(4, 8):
                        brd_piece(0, k, eng=nc.vector)
                    for k in range(3):
                        tanh0_chunk(0, k, h=1)

            # ---- main loop over psum supertiles
            def spread(j):
                """Next-group h0 construction, spread across this group.
                The resid adds live on GpSimd, so the bcast adds can't
                delay the latency-critical tanh1 -> resid -> L2 edge."""
                g, sg = divmod(j, TILES_PER_GROUP)
                if g + 1 >= NGROUPS:
                    return
                if sg <= 1:
                    for k in range(4):
                        brd_piece(g + 1, 4 * sg + k)
                elif 2 <= sg <= 4:
                    tanh0_chunk(g + 1, sg - 2)

            # staged vout drains: window w's columns are final once
            # red(w, 1) has issued (at s2 = 2w + 3)
            drains = {15: (0, 6), 25: (6, 11), 33: (11, 16), 35: (16, 17)}

            for s2 in range(NSUPER):
                l1_mm(2 * s2)
                spread(2 * s2)
                l1_mm(2 * s2 + 1)
                spread(2 * s2 + 1)
                tanh1(s2)
                resid(s2)
                if s2 > 0:
                    l2_mm(s2 - 1)
                    tanh2(s2 - 1)
                # window w complete (tanh2(2w+1) issued) at s2 = 2w+2;
                # stagger t2's reduce one super later so it lands in slack
                if s2 >= 2 and s2 % 2 == 0:
                    red((s2 - 2) // 2, 0)
                if s2 >= 3 and s2 % 2 == 1:
                    red((s2 - 3) // 2, 1)
                if s2 == NSUPER - 1:
                    # first half of the last window is already final: its
                    # tanh1/resid (super 34) and tanh2(34) are in
                    red(NWIN - 1, 0, 0, BPW // 2)
                    red(NWIN - 1, 1, 0, BPW // 2)
                if s2 in drains:
                    w0, w1_ = drains[s2]
                    nc.sync.dma_start(
                        v_d[:, w0 * WCOLS:w1_ * WCOLS],
                        vout[:, w0 * WCOLS:w1_ * WCOLS],
                    )

            l2_mm(NSUPER - 1)
            tanh2(NSUPER - 1)
            red(NWIN - 1, 0, BPW // 2, BPW)
            red(NWIN - 1, 1, BPW // 2, BPW)
            nc.sync.dma_start(
                v_d[:, 17 * WCOLS:], vout[:, 17 * WCOLS:]
            )

    nc.compile()
    _cached[key] = nc
    return nc


def _build_feats(x1, x2):
    """Per-walker first-layer inputs in sign-sorted perm order.

    Returns (X1f [B, 24, 13], X2f [B, 24, 12]): X1f[b, ip] = flattened
    x1[b, P1[_ISORT[ip]]] + trailing 1.0 (carries b0); X2f likewise, no ones.
    """
    B = x1.shape[0]
    xp1 = x1[:, _P1[_ISORT], :].reshape(B, NPERM, N1 * D)
    xp2 = x2[:, _P2[_JSORT], :].reshape(B, NPERM, N2 * D)
    X1f = np.empty((B, NPERM, K1), dtype=np.float32)
    X1f[:, :, :N1 * D] = xp1
    X1f[:, :, N1 * D] = 1.0
    return X1f, np.ascontiguousarray(xp2)


def _make_in_maps(x1, x2, W0, b0, W1, b1, W2, b2):
    with_bias = bool(np.any(b1) or np.any(b2))
    X1f, X2f = _build_feats(x1, x2)
    x1w = np.concatenate([W0[:N1 * D], b0[None, :]], axis=0)  # [13, 256]
    x2w = W0[N1 * D:]  # [12, 256]
    xww = _pack32(x1w, x2w)  # [44, 256]: x1 rows at 0, x2 rows at 32
    wpk = np.ascontiguousarray(np.concatenate(
        [W1[0:128], W1[128:256], W2[0:128], W2[128:256]], axis=1
    ))  # [128, 1024]
    in_maps = []
    for c in range(NCORES):
        sl = slice(c * WALKERS_PER_CORE, (c + 1) * WALKERS_PER_CORE)
        m = {
            "xf": _pack32(X1f[sl].reshape(UCOLS, K1).T,
                          X2f[sl].reshape(UCOLS, K2).T),
            "xw": xww,
            "w": wpk,
        }
        if with_bias:
            bm = np.zeros((128, 4), dtype=np.float32)
            bm[:, 0] = b1[0:128]
            bm[:, 1] = b1[128:256]
            bm[:, 2] = b2[0:128]
            bm[:, 3] = b2[128:256]
            m["b12"] = bm
        in_maps.append(m)
    return with_bias, in_maps


def _pack32(a, b):
    """Stack a (<=32 rows) at partition 0 and b at partition 32."""
    out = np.zeros((32 + b.shape[0], a.shape[1]), dtype=np.float32)
    out[:a.shape[0]] = a
    out[32:] = b
    return out


# static host-side combine map: vout column -> (walker, sign) segment
def _make_combine_map():
    # columns viewed as [w (18), l (2), h (2), b (64)]; block row0 =
    # w*WINROWS + b*BLK; 32-row blocks never straddle walker (576) or
    # sign (288) boundaries.
    w_idx = np.arange(NWIN)[:, None]
    b_idx = np.arange(BPW)[None, :]
    row0 = w_idx * WINROWS + b_idx * BLK          # [18, 64]
    walker = row0 // NPAIR
    sign = (row0 % NPAIR) // HALFPAIR
    seg = (walker * 2 + sign).reshape(-1)          # [1152] per (w, b)
    order = np.argsort(seg, kind="stable")
    starts = np.searchsorted(seg[order], np.arange(2 * WALKERS_PER_CORE))
    return order, starts


_ORDER, _STARTS = _make_combine_map()


def _combine_core(v):
    """vout [128, 4608] -> per-walker signed sums [256, 64] in fp64."""
    v5 = v.reshape(128, NWIN, 2, 2, BPW).astype(np.float64)
    vt = v5.sum(axis=2)                      # [p, w, h, b]  (t1 + t2)
    vt = vt.transpose(2, 0, 1, 3).reshape(2, 128, NWIN * BPW)  # [h, p, wb]
    vs = np.add.reduceat(vt[:, :, _ORDER], _STARTS, axis=2)
    # vs: [h, p, walker*2 + sign]
    d = vs[:, :, 0::2] - vs[:, :, 1::2]      # [h, p, walker]
    return d.reshape(NDENSE, WALKERS_PER_CORE)


def _finish(v_per_core, Wf, bf):
    """per-core vout -> log|anti| [BATCH]."""
    out = np.empty((BATCH,), dtype=np.float32)
    wf64 = Wf[:, 0].astype(np.float64)
    # sum of pair signs is exactly 0, so bf drops out of the signed sum
    for c in range(NCORES):
        u = _combine_core(v_per_core[c])
        anti = wf64 @ u
        out[c * WALKERS_PER_CORE:(c + 1) * WALKERS_PER_CORE] = np.log(
            np.abs(anti)
        ).astype(np.float32)
    return out


def kernel(x1, x2, W0, b0, W1, b1, W2, b2, Wf, bf):
    from concourse.bass_utils import run_bass_kernel_spmd

    x1 = np.asarray(x1, dtype=np.float32)
    x2 = np.asarray(x2, dtype=np.float32)
    W0 = np.asarray(W0, dtype=np.float32)
    b0 = np.asarray(b0, dtype=np.float32)
    W1 = np.asarray(W1, dtype=np.float32)
    b1 = np.asarray(b1, dtype=np.float32)
    W2 = np.asarray(W2, dtype=np.float32)
    b2 = np.asarray(b2, dtype=np.float32)
    Wf = np.asarray(Wf, dtype=np.float32)
    bf = np.asarray(bf, dtype=np.float32)

    with_bias, in_maps = _make_in_maps(x1, x2, W0, b0, W1, b1, W2, b2)
    nc = _build_nc(with_bias)

    res = run_bass_kernel_spmd(nc, in_maps, list(range(NCORES)))
    global _last_results
    _last_results = res

    return _finish([res.results[c]["v"] for c in range(NCORES)], Wf, bf)
